# revision 4
# baseline (speedup 1.0000x reference)
"""BackgroundForegroundNeRF fused MLP kernel for 8x Trainium2 NeuronCores.

Pure data parallel: the 2M points are split across 8 cores; all weights are
replicated. Per core the network runs feature-major ([feature, point] tiles):

  x tile [128pts, 98f] --PE transpose--> xT [98f, npts]
  MM1  : W1 (bg_s0 zero-padded | fg_s0 blocks)       -> h1  [128, n]   relu
  MM2  : block-diag(bg_s1, fg_s1)                    -> h2  [128, n]   relu
  MM3  : sigma/unc head rows only                    -> s   [3, n]     softplus
  C0   : views-part (base-64 zero-padded lhsT) + (c0_geo @ s2_geo) @ h2
         (the geo path is folded into one matrix because there is no
         relu between the sigma-net output and the color-net input)
  C1,C2: block-diag color layers                     relu
  C3   : [bg_color(3) | fg_color(3)]                 -> c3 [6, n]
  PE-transpose s/c3 back to point-major, blend sigma-weighted colors,
  DMA out [n, 6].

Softplus is composed as ln(exp(x) + 1) on the ACT engine (this container's
act tables have no native softplus). All matmuls are fp32 (f32r measured
~0.28 rel err on this hardware - unusable).
"""
import os
import sys

_HERE = os.path.dirname(os.path.abspath(__file__))
sys.path.insert(0, '/opt/trn_rl_repo')

import numpy as np  # noqa: E402

import concourse.bass as bass  # noqa: E402
import concourse.tile as tile  # noqa: E402
from concourse import mybir  # noqa: E402
from concourse.bass_utils import run_bass_kernel_spmd  # noqa: E402

F32 = mybir.dt.float32
AF = mybir.ActivationFunctionType

N_CORES = 8
IN_CH, IN_VIEWS, TIME_DIM, HID, GEO = 71, 27, 8, 64, 15
NF = IN_CH + IN_VIEWS            # 98
TILE_PTS = 2048                  # points per tile iteration
PPB = TILE_PTS // 128            # 16 points per partition
CHUNK = 1024                     # psum chunk (free dim)
MMN = 512                        # matmul moving free dim

LAST_RESULT = None               # BassKernelResults of the last run (for test.py)


def _split_multiwait_instructions(nc, limit=1):
    """The walrus build here rejects instructions with >1 sync wait; hoist
    extra waits onto fresh single-wait NOPs inserted before the instruction."""
    sync_info_cls = None
    for f in nc.m.functions:
        for bb in f.blocks:
            insts = list(bb.instructions)
            if not any(
                i.sync_info is not None and i.sync_info.on_wait
                and len(i.sync_info.on_wait) > limit
                for i in insts
            ):
                continue
            new_list = []
            for inst in insts:
                si = inst.sync_info
                if si is not None and si.on_wait and len(si.on_wait) > limit:
                    if sync_info_cls is None:
                        sync_info_cls = type(si)
                    waits = list(si.on_wait)
                    keep, extra = waits[:limit], waits[limit:]
                    si.on_wait.clear()
                    si.on_wait.extend(keep)
                    for wt in extra:
                        nop = mybir.InstNoOp(
                            name=f"I-mwsplit-{nc.next_id()}", ins=[], outs=[])
                        nop.engine = inst.engine
                        nop.sync_info = sync_info_cls(on_wait=[wt], on_update=[])
                        new_list.append(nop)
                new_list.append(inst)
            while len(bb.instructions):
                bb.instructions.pop()
            for inst in new_list:
                bb.add_instruction(inst)


def _prep_weights(inp):
    """Pack the 14 small MLP weights into fused lhsT ([K, M]) matrices."""
    g = {k: np.asarray(inp[k], np.float32) for k in inp}
    z = np.zeros

    w1 = z((IN_CH, 128), np.float32)            # K=71 -> M=128 (bg|fg h1)
    w1[:63, :64] = g['bg_s0'].T                 # bg uses xyz only (63)
    w1[:71, 64:] = g['fg_s0'].T

    w2 = z((128, 128), np.float32)              # block-diag h1 -> h2
    w2[:64, :64] = g['bg_s1'].T
    w2[64:, 64:] = g['fg_s1'].T

    w3 = z((128, 3), np.float32)                # sigma/unc logits
    w3[:64, 0] = g['bg_s2'][0]                  # bg sigma
    w3[64:, 1] = g['fg_s2'][0]                  # fg sigma
    w3[64:, 2] = g['fg_s2'][1]                  # fg uncertainty

    # c0 views part, padded so lhsT/rhs sit at base partition 64:
    # rows 64..70 (pts tail in xT) are zero, rows 71..97 are the view dirs.
    wc0e = z((NF, 128), np.float32)
    wc0e[71:, :64] = g['bg_c0'][:, :IN_VIEWS].T
    wc0e[71:, 64:] = g['fg_c0'][:, :IN_VIEWS].T

    # c0 geo part folded through the (linear) sigma-net output: geo enters
    # c0 with no relu in between, so c0_geo @ (s2_geo @ h2) collapses.
    bgp = (g['bg_c0'][:, IN_VIEWS:].astype(np.float64)
           @ g['bg_s2'][1:, :].astype(np.float64)).astype(np.float32)
    fgp = (g['fg_c0'][:, IN_VIEWS:].astype(np.float64)
           @ g['fg_s2'][2:, :].astype(np.float64)).astype(np.float32)
    wc0h = z((128, 128), np.float32)
    wc0h[:64, :64] = bgp.T
    wc0h[64:, 64:] = fgp.T

    wc1 = z((128, 128), np.float32)
    wc1[:64, :64] = g['bg_c1'].T
    wc1[64:, 64:] = g['fg_c1'].T
    wc2 = z((128, 128), np.float32)
    wc2[:64, :64] = g['bg_c2'].T
    wc2[64:, 64:] = g['fg_c2'].T

    wc3 = z((128, 6), np.float32)
    wc3[:64, 0:3] = g['bg_c3'].T
    wc3[64:, 3:6] = g['fg_c3'].T

    return {
        'w1': w1, 'w2': w2, 'w3': w3, 'wc0e': wc0e, 'wc0h': wc0h,
        'wc1': wc1, 'wc2': wc2, 'wc3': wc3,
        'ident': np.eye(128, dtype=np.float32),
    }


_PROG_CACHE = {}


def _build_program(padded_pts):
    """Build the per-core Bass program for `padded_pts` points."""
    ntiles = padded_pts // TILE_PTS
    nc = bass.Bass("TRN2", target_bir_lowering=False, debug=False,
                   num_devices=N_CORES)

    xin = nc.dram_tensor("xin", [padded_pts, NF], F32, kind="ExternalInput").ap()
    out = nc.dram_tensor("out", [padded_pts, 6], F32, kind="ExternalOutput").ap()
    wnames = ['w1', 'w2', 'w3', 'wc0e', 'wc0h', 'wc1', 'wc2', 'wc3', 'ident']
    wshapes = {'w1': [IN_CH, 128], 'w2': [128, 128], 'w3': [128, 3],
               'wc0e': [NF, 128], 'wc0h': [128, 128], 'wc1': [128, 128],
               'wc2': [128, 128], 'wc3': [128, 6], 'ident': [128, 128]}
    wdram = {n: nc.dram_tensor(n, wshapes[n], F32, kind="ExternalInput").ap()
             for n in wnames}

    with tile.TileContext(nc) as tc:
        with tc.tile_pool(name="consts", bufs=1) as consts, \
             tc.tile_pool(name="bigs", bufs=2) as bigs, \
             tc.tile_pool(name="io", bufs=3) as io, \
             tc.tile_pool(name="small", bufs=2) as small, \
             tc.tile_pool(name="ps", bufs=4, space="PSUM") as ps:

            W = {}
            for n in wnames:
                W[n] = consts.tile(wshapes[n], F32, name=f"sb_{n}")
                nc.sync.dma_start(out=W[n], in_=wdram[n])
            ident = W['ident']

            for t in range(ntiles):
                rows = slice(t * TILE_PTS, (t + 1) * TILE_PTS)
                x_dram = xin[rows, :].rearrange("(p j) f -> p j f", p=128)
                x_raw = io.tile([128, PPB, NF], F32, name="x_raw", tag="x_raw")
                nc.sync.dma_start(out=x_raw, in_=x_dram)

                xT = bigs.tile([NF, TILE_PTS], F32, name="xT", tag="xT")
                h1r = bigs.tile([128, TILE_PTS], F32, name="h1r", tag="h1r")
                h2r = bigs.tile([128, TILE_PTS], F32, name="h2r", tag="h2r")
                c0r = bigs.tile([128, TILE_PTS], F32, name="c0r", tag="c0r")
                c1r = bigs.tile([128, TILE_PTS], F32, name="c1r", tag="c1r")
                c2r = bigs.tile([128, TILE_PTS], F32, name="c2r", tag="c2r")
                s3 = bigs.tile([3, TILE_PTS], F32, name="s3", tag="s3")
                t6 = bigs.tile([6, TILE_PTS], F32, name="t6", tag="t6")

                nchunk = TILE_PTS // CHUNK
                for ch in range(nchunk):
                    gsl = slice(ch * CHUNK, (ch + 1) * CHUNK)
                    gpb = CHUNK // 128  # transpose groups per chunk

                    # --- transpose x to feature-major ---
                    p_xt = ps.tile([NF, CHUNK], F32, name="p_xt", tag="ps")
                    for j2 in range(gpb):
                        j = ch * gpb + j2
                        nc.tensor.transpose(
                            out=p_xt[:, j2 * 128:(j2 + 1) * 128],
                            in_=x_raw[:, j, :], identity=ident)
                    nc.vector.tensor_copy(out=xT[:, gsl], in_=p_xt)

                    # --- h1 = relu(W1 @ pts) ---
                    p_h1 = ps.tile([128, CHUNK], F32, name="p_h1", tag="ps")
                    for s in range(CHUNK // MMN):
                        msl = slice(s * MMN, (s + 1) * MMN)
                        nc.tensor.matmul(p_h1[:, msl], W['w1'],
                                         xT[0:IN_CH, gsl][:, msl],
                                         start=True, stop=True)
                    nc.scalar.activation(out=h1r[:, gsl], in_=p_h1, func=AF.Relu)

                    # --- h2 = relu(W2 @ h1) ---
                    p_h2 = ps.tile([128, CHUNK], F32, name="p_h2", tag="ps")
                    for s in range(CHUNK // MMN):
                        msl = slice(s * MMN, (s + 1) * MMN)
                        nc.tensor.matmul(p_h2[:, msl], W['w2'],
                                         h1r[:, gsl][:, msl],
                                         start=True, stop=True)
                    nc.scalar.activation(out=h2r[:, gsl], in_=p_h2, func=AF.Relu)

                    # --- sigma/unc logits + softplus = ln(exp(x)+1) ---
                    p_s = ps.tile([3, CHUNK], F32, name="p_s", tag="ps")
                    for s in range(CHUNK // MMN):
                        msl = slice(s * MMN, (s + 1) * MMN)
                        nc.tensor.matmul(p_s[:, msl], W['w3'],
                                         h2r[:, gsl][:, msl],
                                         start=True, stop=True)
                    nc.scalar.activation(out=s3[:, gsl], in_=p_s, func=AF.Exp)
                    nc.scalar.activation(out=s3[:, gsl], in_=s3[:, gsl],
                                         func=AF.Ln, bias=1.0)

                    # --- c0 = relu(Wc0e @ [pts_tail|views] + Wc0h @ h2) ---
                    p_c0 = ps.tile([128, CHUNK], F32, name="p_c0", tag="ps")
                    for s in range(CHUNK // MMN):
                        msl = slice(s * MMN, (s + 1) * MMN)
                        nc.tensor.matmul(p_c0[:, msl], W['wc0e'][64:NF, :],
                                         xT[64:NF, gsl][:, msl],
                                         start=True, stop=False)
                        nc.tensor.matmul(p_c0[:, msl], W['wc0h'],
                                         h2r[:, gsl][:, msl],
                                         start=False, stop=True)
                    nc.scalar.activation(out=c0r[:, gsl], in_=p_c0, func=AF.Relu)

                    # --- c1, c2 (relu on DVE) ---
                    p_c1 = ps.tile([128, CHUNK], F32, name="p_c1", tag="ps")
                    for s in range(CHUNK // MMN):
                        msl = slice(s * MMN, (s + 1) * MMN)
                        nc.tensor.matmul(p_c1[:, msl], W['wc1'],
                                         c0r[:, gsl][:, msl],
                                         start=True, stop=True)
                    nc.vector.tensor_scalar_max(c1r[:, gsl], p_c1, 0.0)

                    p_c2 = ps.tile([128, CHUNK], F32, name="p_c2", tag="ps")
                    for s in range(CHUNK // MMN):
                        msl = slice(s * MMN, (s + 1) * MMN)
                        nc.tensor.matmul(p_c2[:, msl], W['wc2'],
                                         c1r[:, gsl][:, msl],
                                         start=True, stop=True)
                    nc.vector.tensor_scalar_max(c2r[:, gsl], p_c2, 0.0)

                    # --- c3 = [bg_color | fg_color] ---
                    p_c3 = ps.tile([6, CHUNK], F32, name="p_c3", tag="ps")
                    for s in range(CHUNK // MMN):
                        msl = slice(s * MMN, (s + 1) * MMN)
                        nc.tensor.matmul(p_c3[:, msl], W['wc3'],
                                         c2r[:, gsl][:, msl],
                                         start=True, stop=True)
                    nc.vector.tensor_copy(out=t6[:, gsl], in_=p_c3)

                # --- back to point-major: P9[p, j*9+c] ---
                p9 = ps.tile([128, PPB * 9], F32, name="p9", tag="ps")
                for j in range(PPB):
                    csl = slice(j * 128, (j + 1) * 128)
                    nc.tensor.transpose(out=p9[:, j * 9:j * 9 + 6],
                                        in_=t6[:, csl],
                                        identity=ident[0:6, 0:6])
                    nc.tensor.transpose(out=p9[:, j * 9 + 6:j * 9 + 9],
                                        in_=s3[:, csl],
                                        identity=ident[0:3, 0:3])
                p9r = p9.rearrange("p (j c) -> p j c", c=9)

                out_sb = io.tile([128, PPB, 6], F32, name="out_sb", tag="out_sb")
                p9s = small.tile([128, PPB, 9], F32, name="p9s", tag="p9s")
                sig = small.tile([128, PPB], F32, name="sig", tag="sig")
                inv = small.tile([128, PPB], F32, name="inv", tag="inv")
                wbg = small.tile([128, PPB], F32, name="wbg", tag="wbg")
                wfg = small.tile([128, PPB], F32, name="wfg", tag="wfg")
                cbg = small.tile([128, PPB, 3], F32, name="cbg", tag="cbg")
                cfg = small.tile([128, PPB, 3], F32, name="cfg", tag="cfg")

                nc.vector.tensor_copy(out=p9s, in_=p9r)
                nc.vector.tensor_add(sig, p9s[:, :, 6], p9s[:, :, 7])
                nc.vector.tensor_scalar_add(out_sb[:, :, 3], sig, 1e-9)
                nc.vector.reciprocal(out=inv, in_=out_sb[:, :, 3])
                nc.vector.tensor_mul(wbg, p9s[:, :, 6], inv)
                nc.vector.tensor_mul(wfg, p9s[:, :, 7], inv)
                nc.vector.tensor_mul(
                    cbg, p9s[:, :, 0:3],
                    wbg.unsqueeze(2).broadcast_to((128, PPB, 3)))
                nc.vector.tensor_mul(
                    cfg, p9s[:, :, 3:6],
                    wfg.unsqueeze(2).broadcast_to((128, PPB, 3)))
                nc.vector.tensor_add(out_sb[:, :, 0:3], cbg, cfg)
                nc.vector.tensor_copy(out=out_sb[:, :, 4], in_=p9s[:, :, 8])
                nc.vector.tensor_copy(out=out_sb[:, :, 5], in_=p9s[:, :, 7])

                o_dram = out[rows, :].rearrange("(p j) f -> p j f", p=128)
                nc.sync.dma_start(out=o_dram, in_=out_sb)

    _split_multiwait_instructions(nc)
    return nc


def kernel(**inputs):
    global LAST_RESULT
    x = np.ascontiguousarray(np.asarray(inputs['x'], dtype=np.float32))
    n_total = x.shape[0]
    per_core = (n_total + N_CORES - 1) // N_CORES
    ntiles = (per_core + TILE_PTS - 1) // TILE_PTS
    padded = ntiles * TILE_PTS

    key = padded
    if key not in _PROG_CACHE:
        _PROG_CACHE[key] = _build_program(padded)
    nc = _PROG_CACHE[key]

    w = _prep_weights({k: v for k, v in inputs.items() if k != 'x'})

    in_maps = []
    for c in range(N_CORES):
        lo = c * per_core
        hi = min(lo + per_core, n_total)
        xc = np.zeros((padded, NF), np.float32)
        xc[:hi - lo] = x[lo:hi]
        in_maps.append({'xin': xc, **w})

    trace = bool(int(os.environ.get('NERF_TRACE', '0')))
    res = run_bass_kernel_spmd(nc, in_maps, list(range(N_CORES)), trace=trace)
    LAST_RESULT = res

    pieces = []
    for c in range(N_CORES):
        lo = c * per_core
        hi = min(lo + per_core, n_total)
        pieces.append(res.results[c]['out'][:hi - lo])
    return np.concatenate(pieces, axis=0)
